# revision 1
# baseline (speedup 1.0000x reference)
"""Distributed CL loss kernel for Trainium2 (8 NeuronCores).

Reference computes  mean_i sum_j ||s_i - t_j||^2 * [tg_i == tg_j] / cnt[tg_i]
with the [N, N] pairwise-distance matrix.  Because the mask only depends on
the class labels, the whole loss collapses to per-class aggregates:

  sum_j d2[i,j]*mask[i,j] = cnt[c_i]*|s_i|^2 + sum_{j in c_i}|t_j|^2
                            - 2 * s_i . T_{c_i}
  loss = (1/N) * [ sum_i |s_i|^2 + sum_j |t_j|^2 - 2 * sum_c S_c.T_c / cnt_c ]

with S_c / T_c the class-sums of fm_s / fm_t rows.  So the device work is a
single streaming pass: class-sum matmuls (one-hot^T @ X on the PE, fp32r
single-pass) plus sum-of-squares reductions (fm_s on ACT via Square+accum,
fm_t on DVE via mul+reduce), sharded by rows across the 8 cores.  The
remaining O(C*D) combine runs on the host while gathering.

fp32r notes: matmul inputs are declared float32r (same f32 bits; the PE
streams them single-pass at ~TF32 effective precision, 4x faster than the
fp32 two-pass path).  That reduced precision only touches the class-sum
cross term, which contributes ~0.1% of the loss magnitude — measured final
relative error stays ~1e-6.  The sum-of-squares paths read the same SBUF
bytes bitcast back to plain f32, so the dominant |s|^2+|t|^2 terms keep
full fp32 precision.
"""

import numpy as np

N, D, NUM_CLASSES = 4096, 1024, 10
NCORES = 8
RPC = N // NCORES  # rows per core (both fm_s and fm_t are row-sharded)
KT = RPC // 128    # 128-row k-tiles per core
CP = 16            # class dim padded for alignment
DW = D + CP        # raw kernel tile width: data + appended one-hot

_STATE = {}
LAST_RUN = None  # BassKernelResults of the most recent device run (for test.py)


def _build_nc_tile():
    import concourse.bacc as bacc
    import concourse.mybir as mybir
    import concourse.tile as tile

    f32 = mybir.dt.float32
    f32r = mybir.dt.float32r
    nc = bacc.Bacc(
        "TRN2",
        target_bir_lowering=False,
        debug=False,
        enable_asserts=False,
        num_devices=NCORES,
    )

    s_in = nc.dram_tensor("s_in", (RPC, D), f32r, kind="ExternalInput")
    t_in = nc.dram_tensor("t_in", (RPC, D), f32r, kind="ExternalInput")
    oh_in = nc.dram_tensor("oh_in", (RPC, CP), f32r, kind="ExternalInput")
    S_out = nc.dram_tensor("S_out", (CP, D), f32, kind="ExternalOutput")
    T_out = nc.dram_tensor("T_out", (CP, D), f32, kind="ExternalOutput")
    st_out = nc.dram_tensor("st_out", (128, 2 * KT), f32, kind="ExternalOutput")

    # row r = n*128 + p  ->  partition p, k-tile n
    s_r = s_in.ap().rearrange("(n p) d -> p n d", p=128)
    t_r = t_in.ap().rearrange("(n p) d -> p n d", p=128)
    oh_r = oh_in.ap().rearrange("(n p) c -> p n c", p=128)

    with tile.TileContext(nc) as tc:
        with (
            tc.tile_pool(name="data", bufs=KT) as data_pool,
            tc.tile_pool(name="scratch", bufs=2) as scratch_pool,
            tc.tile_pool(name="small", bufs=1) as small_pool,
            tc.tile_pool(name="psum", bufs=1, space="PSUM") as psum_pool,
        ):
            oh_sb = small_pool.tile([128, KT, CP], f32r, tag="oh")
            nc.sync.dma_start(oh_sb[:], oh_r)
            stats = small_pool.tile([128, 2 * KT], f32, tag="stats")

            pS0 = psum_pool.tile([CP, 512], f32, tag="pS0")
            pS1 = psum_pool.tile([CP, 512], f32, tag="pS1")
            pT0 = psum_pool.tile([CP, 512], f32, tag="pT0")
            pT1 = psum_pool.tile([CP, 512], f32, tag="pT1")

            for k in range(KT):
                start, stop = k == 0, k == KT - 1
                s_t = data_pool.tile([128, D], f32r, tag="s")
                nc.sync.dma_start(s_t[:], s_r[:, k, :])
                t_t = data_pool.tile([128, D], f32r, tag="t")
                nc.gpsimd.dma_start(t_t[:], t_r[:, k, :])
                oh_k = oh_sb[:, k, :]

                nc.tensor.matmul(pS0[:], oh_k, s_t[:, 0:512], start=start, stop=stop)
                nc.tensor.matmul(pS1[:], oh_k, s_t[:, 512:D], start=start, stop=stop)
                nc.tensor.matmul(pT0[:], oh_k, t_t[:, 0:512], start=start, stop=stop)
                nc.tensor.matmul(pT1[:], oh_k, t_t[:, 512:D], start=start, stop=stop)

                # |s|^2 on ACT: fused square + free-axis accumulate
                sq_s = scratch_pool.tile([128, D], f32, tag="sq_s")
                nc.scalar.activation(
                    sq_s[:],
                    s_t[:].bitcast(f32),
                    mybir.ActivationFunctionType.Square,
                    accum_out=stats[:, k : k + 1],
                )
                # |t|^2 on DVE: square then reduce (tensor_tensor_reduce
                # mis-executes on HW, so two plain ops)
                sq_t = scratch_pool.tile([128, D], f32, tag="sq_t")
                nc.vector.tensor_mul(
                    sq_t[:], t_t[:].bitcast(f32), t_t[:].bitcast(f32)
                )
                nc.vector.reduce_sum(
                    stats[:, KT + k : KT + k + 1],
                    sq_t[:],
                    axis=mybir.AxisListType.X,
                )

            S_sb = small_pool.tile([CP, D], f32, tag="S_sb")
            T_sb = small_pool.tile([CP, D], f32, tag="T_sb")
            nc.scalar.copy(S_sb[:, 0:512], pS0[:])
            nc.scalar.copy(S_sb[:, 512:D], pS1[:])
            nc.vector.tensor_copy(T_sb[:, 0:512], pT0[:])
            nc.vector.tensor_copy(T_sb[:, 512:D], pT1[:])

            nc.sync.dma_start(S_out.ap(), S_sb[:])
            nc.sync.dma_start(T_out.ap(), T_sb[:])
            nc.sync.dma_start(st_out.ap(), stats[:])

    nc.compile()
    return nc


def build_nc_raw():
    import concourse.bacc as bacc
    import concourse.mybir as mybir

    f32 = mybir.dt.float32
    f16 = mybir.dt.float16
    nc = bacc.Bacc(
        "TRN2",
        target_bir_lowering=False,
        debug=False,
        enable_asserts=False,
        num_devices=NCORES,
    )

    s_in = nc.dram_tensor("s_in", (RPC, DW), f16, kind="ExternalInput")
    t_in = nc.dram_tensor("t_in", (RPC, DW), f16, kind="ExternalInput")
    S_out = nc.dram_tensor("S_out", (CP, D), f32, kind="ExternalOutput")
    T_out = nc.dram_tensor("T_out", (CP, D), f32, kind="ExternalOutput")
    st_out = nc.dram_tensor("st_out", (128, 2 * KT), f32, kind="ExternalOutput")

    s_r = s_in.ap().rearrange("(n p) d -> p n d", p=128)
    t_r = t_in.ap().rearrange("(n p) d -> p n d", p=128)

    s_sb = nc.alloc_sbuf_tensor("s_sb", [128, KT, DW], f16)
    t_sb = nc.alloc_sbuf_tensor("t_sb", [128, KT, DW], f16)
    sq_s = nc.alloc_sbuf_tensor("sq_s", [128, 2, D], f32)
    sq_t = nc.alloc_sbuf_tensor("sq_t", [128, 2, D], f16)
    stats = nc.alloc_sbuf_tensor("stats", [128, 2 * KT], f32)
    S_sb = nc.alloc_sbuf_tensor("S_sb", [CP, D], f32)
    T_sb = nc.alloc_sbuf_tensor("T_sb", [CP, D], f32)

    pS = [nc.alloc_psum_tensor(f"pS{h}", [CP, 512], f32) for h in range(2)]
    pT = [nc.alloc_psum_tensor(f"pT{h}", [CP, 512], f32) for h in range(2)]

    s_sems = [nc.alloc_semaphore(f"s_sem{k}") for k in range(KT)]
    t_sems = [nc.alloc_semaphore(f"t_sem{k}") for k in range(KT)]
    pSd = [nc.alloc_semaphore(f"pS{h}d") for h in range(2)]
    pTd = [nc.alloc_semaphore(f"pT{h}d") for h in range(2)]
    act_done = nc.alloc_semaphore("act_done")
    dve_done = nc.alloc_semaphore("dve_done")
    dve_mul = nc.alloc_semaphore("dve_mul")
    s_copy = nc.alloc_semaphore("s_copy")
    t_copy = nc.alloc_semaphore("t_copy")
    out_sem = nc.alloc_semaphore("out_sem")
    stats_sem = nc.alloc_semaphore("stats_sem")

    Sq = mybir.ActivationFunctionType.Square
    X = mybir.AxisListType.X

    # queue -> FIFO tile lists (which, k)
    q_sync = [("s", 0), ("t", 1), ("s", 3)]
    q_scal = [("t", 0), ("t", 2), ("s", 2)]
    q_gps = [("s", 1), ("t", 3)]
    # PE order: round-robin across queues in FIFO position
    pe_order = [("s", 0), ("t", 0), ("s", 1), ("t", 1), ("t", 2), ("t", 3), ("s", 3), ("s", 2)]
    # squares: ACT the s tiles, DVE the t tiles (fp16 scratch -> 2x mode)
    act_tiles = [("s", 0), ("s", 1), ("s", 3), ("s", 2)]
    dve_tiles = [("t", 0), ("t", 1), ("t", 2), ("t", 3)]

    def tile_parts(which, k):
        if which == "s":
            return s_sems[k], s_sb, s_r
        return t_sems[k], t_sb, t_r

    with nc.Block() as block:

        def issue(engine, tiles):
            for which, k in tiles:
                sem, sb, r = tile_parts(which, k)
                engine.dma_start(sb[:, k, :], r[:, k, :]).then_inc(sem, 16)

        @block.sync
        def _(sync):
            issue(sync, q_sync)
            sync.wait_ge(t_copy, 2)
            sync.dma_start(T_out.ap(), T_sb[:]).then_inc(out_sem, 16)
            sync.wait_ge(out_sem, 32)
            sync.wait_ge(stats_sem, 16)

        @block.gpsimd
        def _(gpsimd):
            issue(gpsimd, q_gps)
            gpsimd.wait_ge(act_done, len(act_tiles))
            gpsimd.wait_ge(dve_done, len(dve_tiles))
            gpsimd.dma_start(st_out.ap(), stats[:]).then_inc(stats_sem, 16)

        @block.tensor
        def _(tensor):
            n_seen = {"s": 0, "t": 0}
            for which, k in pe_order:
                sem, sb, _ = tile_parts(which, k)
                banks, dsems = (pS, pSd) if which == "s" else (pT, pTd)
                n_seen[which] += 1
                start = n_seen[which] == 1
                stop = n_seen[which] == KT
                tensor.wait_ge(sem, 16)
                oh_k = sb[:, k, D:DW]
                for h in range(2):
                    mm = tensor.matmul(
                        banks[h][:],
                        oh_k,
                        sb[:, k, 512 * h : 512 * (h + 1)],
                        start=start,
                        stop=stop,
                    )
                    if stop:
                        mm.then_inc(dsems[h], 1)

        @block.scalar
        def _(scalar):
            issue(scalar, q_scal)
            for i, (w, k) in enumerate(act_tiles):
                sem, sb, _ = tile_parts(w, k)
                col = k if w == "s" else KT + k
                scalar.wait_ge(sem, 16)
                if i >= 2:
                    # scratch buffer i%2 free once square i-2 fully retired
                    scalar.wait_ge(act_done, i - 1)
                scalar.activation(
                    sq_s[:, i % 2, :],
                    sb[:, k, 0:D],
                    Sq,
                    accum_out=stats[:, col : col + 1],
                ).then_inc(act_done, 1)
            for h in range(2):
                scalar.wait_ge(pSd[h], 1)
                scalar.copy(S_sb[:, 512 * h : 512 * (h + 1)], pS[h][:]).then_inc(
                    s_copy, 1
                )
            scalar.wait_ge(s_copy, 2)
            scalar.dma_start(S_out.ap(), S_sb[:]).then_inc(out_sem, 16)

        @block.vector
        def _(vector):
            for i, (w, k) in enumerate(dve_tiles):
                sem, sb, _ = tile_parts(w, k)
                vector.wait_ge(sem, 16)
                if i >= 2:
                    vector.wait_ge(dve_done, i - 1)
                vector.tensor_mul(
                    sq_t[:, i % 2, :],
                    sb[:, k, 0:D],
                    sb[:, k, 0:D],
                ).then_inc(dve_mul, 1)
                vector.wait_ge(dve_mul, i + 1)
                vector.reduce_sum(
                    stats[:, KT + k : KT + k + 1], sq_t[:, i % 2, :], axis=X
                ).then_inc(dve_done, 1)
            for h in range(2):
                vector.wait_ge(pTd[h], 1)
                vector.tensor_copy(T_sb[:, 512 * h : 512 * (h + 1)], pT[h][:]).then_inc(
                    t_copy, 1
                )

    nc.compile()
    return nc


def _build_nc():
    import os
    if os.environ.get("KERNEL_IMPL", "raw") == "tile":
        return _build_nc_tile()
    return build_nc_raw()


def _get_nc():
    if "nc" not in _STATE:
        _STATE["nc"] = _build_nc()
    return _STATE["nc"]


def kernel(fm_s, fm_t, targets, fusion_true=0, **_unused):
    global LAST_RUN
    from concourse.bass_utils import run_bass_kernel_spmd

    fm_s = np.ascontiguousarray(np.asarray(fm_s, dtype=np.float32))
    fm_t = np.ascontiguousarray(np.asarray(fm_t, dtype=np.float32))
    tg = np.asarray(targets).astype(np.int64).ravel()
    assert fm_s.shape == (N, D) and fm_t.shape == (N, D) and tg.shape == (N,)

    oh = (tg[:, None] == np.arange(CP, dtype=np.int64)[None, :]).astype(np.float32)
    counts = np.bincount(tg, minlength=CP).astype(np.float64)[:CP]
    # append the one-hot columns to every row so each 128-row tile DMA is
    # self-contained (the PE takes lhsT from the tile's own tail columns)
    s_aug = np.concatenate([fm_s, oh], axis=1).astype(np.float16)
    t_aug = np.concatenate([fm_t, oh], axis=1).astype(np.float16)

    in_maps = [
        {
            "s_in": s_aug[c * RPC : (c + 1) * RPC],
            "t_in": t_aug[c * RPC : (c + 1) * RPC],
        }
        for c in range(NCORES)
    ]

    nc = _get_nc()
    LAST_RUN = run_bass_kernel_spmd(nc, in_maps, list(range(NCORES)))
    res = LAST_RUN.results

    S = np.zeros((CP, D), np.float64)
    T = np.zeros((CP, D), np.float64)
    ss = 0.0
    tt = 0.0
    for r in res:
        S += r["S_out"].astype(np.float64)
        T += r["T_out"].astype(np.float64)
        ss += float(r["st_out"][:, :KT].astype(np.float64).sum())
        tt += float(r["st_out"][:, KT:].astype(np.float64).sum())

    safe = np.where(counts > 0, counts, 1.0)
    dot = float(((S * T).sum(axis=1) / safe).sum())
    loss = (ss + tt - 2.0 * dot) / N
    return np.array(loss, dtype=np.float32)



# revision 13
# speedup vs baseline: 1.1702x; 1.1702x over previous
"""Distributed CL loss kernel for Trainium2 (8 NeuronCores).

Reference computes  mean_i sum_j ||s_i - t_j||^2 * [tg_i == tg_j] / cnt[tg_i].
Because the mask depends only on class labels, the loss collapses to

  loss = (1/N) * [ sum|s|^2 + sum|t|^2 - 2 * sum_c S_c.T_c / cnt_c ]

with S_c/T_c the class-sums of fm_s/fm_t rows.  Device work per core (rows
sharded 512/core) is one streaming pass over the data:

  * class sums on the PE:  one-hot^T @ X as fp8e4 DoubleRow matmuls
    (256-row contraction, 2 fp8 weights per PE cell, 0.5 cyc/col)
  * sum-of-squares split across ACT (Square activation with accum_out) and
    DVE + GpSimd (fused scalar_tensor_tensor x*x with accum_out), sliced by
    column so every engine chews each arriving chunk in parallel

fp8e4 (TRN E4M3, max 240) end-to-end measures ~7e-4 relative error vs the
fp32 reference - the quantization bias on the dominant |x|^2 terms.

Host packs rows so each partition's bytes are contiguous in DRAM (row r of a
core maps to tile r//256, ko (r%256)//128, partition r%128; line = 1024 data
+ 16 one-hot + 16 pad fp8 bytes).  Four chunk DMAs (s-tile0, s-tile1,
t-tile0, t-tile1) stream on a single queue so completions are in-order on
one semaphore; compute chases the DMA ladder.  Outputs are one PSUM->DRAM
DMA ([16, 4, 512] class sums) and one stats DMA ([128, 16] accumulators).
"""

import os

import numpy as np

N, D, NUM_CLASSES = 4096, 1024, 10
NCORES = 8
RPC = N // NCORES  # 512 rows per core
CP = 16            # class columns padded for alignment
PAD = 16
LINE = D + CP + PAD  # 1056 fp8 bytes per ko-row
NT = 2             # DoubleRow tiles per tensor per core (256 rows each)

# column split of the square pass: ACT / DVE / GpSimd
CA, CV = 489, 535
assert CA + CV == D

_STATE = {}
LAST_RUN = None  # BassKernelResults of the most recent device run (for test.py)


def build_nc():
    import concourse.bacc as bacc
    import concourse.mybir as mybir

    f32 = mybir.dt.float32
    f16 = mybir.dt.float16
    f8 = mybir.dt.float8e4
    mult = mybir.AluOpType.mult
    Sq = mybir.ActivationFunctionType.Square
    DR = mybir.MatmulPerfMode.DoubleRow

    mm_mode = os.environ.get("KERNEL_MM", "dr")      # dr | flat
    sq_mode = os.environ.get("KERNEL_SQ", "stt")     # stt | mulred
    out_mode = "copy"

    nc = bacc.Bacc(
        "TRN2",
        target_bir_lowering=False,
        debug=False,
        enable_asserts=False,
        num_devices=NCORES,
    )

    s_in = nc.dram_tensor("s_in", (128, NT, 2, LINE), f8, kind="ExternalInput")
    t_in = nc.dram_tensor("t_in", (128, NT, 2, LINE), f8, kind="ExternalInput")
    st_out = nc.dram_tensor("st_out", (CP, 4, 512), f32, kind="ExternalOutput")
    stats_out = nc.dram_tensor("stats_out", (128, 8), f32, kind="ExternalOutput")

    s_sb = nc.alloc_sbuf_tensor("s_sb", [128, NT, 2, LINE], f8)
    t_sb = nc.alloc_sbuf_tensor("t_sb", [128, NT, 2, LINE], f8)
    stats = nc.alloc_sbuf_tensor("stats", [128, 8], f32)
    sq_a = nc.alloc_sbuf_tensor("sq_a", [128, 4, 2, CA], f16)
    sq_v = nc.alloc_sbuf_tensor("sq_v", [128, 4, 2, CV], f16)
    st_sb = nc.alloc_sbuf_tensor("st_sb", [CP, 4, 512], f32)

    pAll = nc.alloc_psum_tensor("pAll", [CP, 4, 512], f32)

    sem_in = [nc.alloc_semaphore(f"sem_in{i}") for i in range(4)]
    sem_pe = nc.alloc_semaphore("sem_pe")
    sem_cp = nc.alloc_semaphore("sem_cp")
    sem_sq = nc.alloc_semaphore("sem_sq")
    sem_out = nc.alloc_semaphore("sem_out")
    sem_out2 = nc.alloc_semaphore("sem_out2")

    CHUNKS = [("s", 0), ("s", 1), ("t", 0), ("t", 1)]

    def sb(which):
        return s_sb if which == "s" else t_sb

    def din(which):
        return s_in if which == "s" else t_in

    with nc.Block() as block:

        @block.sync
        def _(sync):
            for i, (w, T) in enumerate(CHUNKS):
                sync.dma_start(sb(w)[:, T], din(w).ap()[:, T]).then_inc(
                    sem_in[i], 16
                )
            sync.wait_ge(sem_cp, 4)
            sync.dma_start(st_out.ap(), st_sb[:]).then_inc(sem_out, 16)
            sync.wait_ge(sem_out, 16)
            sync.wait_ge(sem_out2, 16)

        @block.tensor
        def _(tensor):
            for i, (w, T) in enumerate(CHUNKS):
                tensor.wait_ge(sem_in[i], 16)
                x = sb(w)
                start, stop = T == 0, T == 1
                for h in range(2):
                    bank = (0 if w == "s" else 2) + h
                    if mm_mode == "dr":
                        mm = tensor.matmul(
                            pAll[:, bank, :],
                            x[:, T, :, D : D + CP],
                            x[:, T, :, 512 * h : 512 * (h + 1)],
                            start=start,
                            stop=stop,
                            perf_mode=DR,
                        )
                        if stop:
                            mm.then_inc(sem_pe, 1)
                    else:
                        for ko in range(2):
                            mm = tensor.matmul(
                                pAll[:, bank, :],
                                x[:, T, ko, D : D + CP],
                                x[:, T, ko, 512 * h : 512 * (h + 1)],
                                start=start and ko == 0,
                                stop=stop and ko == 1,
                            )
                            if stop and ko == 1:
                                mm.then_inc(sem_pe, 1)

        @block.scalar
        def _(scalar):
            for i, (w, T) in enumerate(CHUNKS):
                scalar.wait_ge(sem_in[i], 16)
                a = scalar.activation(
                    sq_a[:, i],
                    sb(w)[:, T, :, 0:CA],
                    Sq,
                    accum_out=stats[:, i : i + 1],
                )
                if i == 3:
                    a.then_inc(sem_sq, 1)
            for b in range(2):
                scalar.wait_ge(sem_pe, 2)
                scalar.copy(st_sb[:, b, :], pAll[:, b, :]).then_inc(sem_cp, 1)

        def squares(engine, scratch, c0, c1, col_base, fused):
            for i, (w, T) in enumerate(CHUNKS):
                engine.wait_ge(sem_in[i], 16)
                src = sb(w)[:, T, :, c0:c1]
                col = stats[:, col_base + i : col_base + i + 1]
                if fused:
                    op = engine.scalar_tensor_tensor(
                        scratch[:, i], src, 1.0, src, mult, mult, accum_out=col
                    )
                else:
                    engine.tensor_mul(scratch[:, i], src, src)
                    op = engine.reduce_sum(
                        col, scratch[:, i], axis=mybir.AxisListType.X
                    )
                if i == 3:
                    op.then_inc(sem_sq, 1)

        @block.vector
        def _(vector):
            squares(vector, sq_v, CA, D, 4, sq_mode == "stt")
            for b in range(2, 4):
                vector.wait_ge(sem_pe, 4)
                vector.tensor_copy(st_sb[:, b, :], pAll[:, b, :]).then_inc(
                    sem_cp, 1
                )

        @block.gpsimd
        def _(gpsimd):
            # Pool cannot run TensorScalarPtr and cannot read PSUM; it just
            # ships the stats once both square engines have finished.
            gpsimd.wait_ge(sem_sq, 2)
            gpsimd.dma_start(stats_out.ap(), stats[:]).then_inc(sem_out2, 16)

    nc.compile()
    return nc


def _get_nc():
    if "nc" not in _STATE:
        _STATE["nc"] = build_nc()
    return _STATE["nc"]


def _f8():
    import ml_dtypes

    return ml_dtypes.float8_e4m3


def pack_inputs(fm_s, fm_t, targets):
    """fp8-quantize, append one-hot columns, and lay rows out so each
    partition's bytes are contiguous in DRAM: [core, 128, NT, 2, LINE]."""
    f8 = _f8()
    tg = np.asarray(targets).astype(np.int64).ravel()
    oh = (tg[:, None] == np.arange(CP, dtype=np.int64)[None, :]).astype(f8)

    def pack(x):
        aug = np.zeros((N, LINE), dtype=f8)
        aug[:, :D] = np.asarray(x, dtype=np.float32).astype(f8)
        aug[:, D : D + CP] = oh
        per = aug.reshape(NCORES, NT, 2, 128, LINE).transpose(0, 3, 1, 2, 4)
        return np.ascontiguousarray(per)

    counts = np.bincount(tg, minlength=CP).astype(np.float64)[:CP]
    return pack(fm_s), pack(fm_t), counts


def kernel(fm_s, fm_t, targets, fusion_true=0, **_unused):
    global LAST_RUN
    from concourse.bass_utils import run_bass_kernel_spmd

    s_pack, t_pack, counts = pack_inputs(fm_s, fm_t, targets)

    in_maps = [
        {"s_in": s_pack[c], "t_in": t_pack[c]} for c in range(NCORES)
    ]

    nc = _get_nc()
    LAST_RUN = run_bass_kernel_spmd(nc, in_maps, list(range(NCORES)))
    res = LAST_RUN.results

    S = np.zeros((CP, D), np.float64)
    T = np.zeros((CP, D), np.float64)
    sq = 0.0
    for r in res:
        st = r["st_out"].astype(np.float64)
        S += st[:, 0:2, :].reshape(CP, D)
        T += st[:, 2:4, :].reshape(CP, D)
        sq += float(r["stats_out"].astype(np.float64).sum())

    safe = np.where(counts > 0, counts, 1.0)
    dot = float(((S * T).sum(axis=1) / safe).sum())
    loss = (sq - 2.0 * dot) / N
    return np.array(loss, dtype=np.float32)
